# revision 2
# baseline (speedup 1.0000x reference)
"""AxialBlock kernel for trn2 NeuronCores (axon jax backend).

The whole block (convs + two axial-attention blocks + batchnorms) is
expressed as a single XLA module and compiled for the Neuron backend once
at import time (AOT). `kernel()` then only pays device transfer +
execution, which is what the harness times.

Shapes are hardcoded per the problem spec: x [16,128,64,64] f32;
MID=128, COUT=256, G=8 groups, GP=16 planes/group, K=64.
"""

import os

import numpy as np

NB, CIN, COUT, MID, G, K = 16, 128, 256, 128, 8, 64
GP = MID // G  # 16
EPS = 1e-5

_INPUT_SPECS = [
    ("x", (NB, CIN, K, K)),
    ("c1_w", (COUT, CIN)), ("c1_b", (COUT,)),
    ("cd_w", (MID, CIN)), ("cd_b", (MID,)),
    ("bn1_g", (MID,)), ("bn1_b", (MID,)),
    ("h_qkv_w", (2 * MID, MID)),
    ("h_bq_g", (2 * MID,)), ("h_bq_b", (2 * MID,)),
    ("h_bs_g", (3 * G,)), ("h_bs_b", (3 * G,)),
    ("h_bo_g", (2 * MID,)), ("h_bo_b", (2 * MID,)),
    ("h_rel", (2 * GP, 2 * K - 1)),
    ("w_qkv_w", (2 * MID, MID)),
    ("w_bq_g", (2 * MID,)), ("w_bq_b", (2 * MID,)),
    ("w_bs_g", (3 * G,)), ("w_bs_b", (3 * G,)),
    ("w_bo_g", (2 * MID,)), ("w_bo_b", (2 * MID,)),
    ("w_rel", (2 * GP, 2 * K - 1)),
    ("cu_w", (COUT, MID)), ("cu_b", (COUT,)),
    ("bn2_g", (COUT,)), ("bn2_b", (COUT,)),
]

# ---------------------------------------------------------------------------
# jax implementation (one fused module)
# ---------------------------------------------------------------------------

_compiled = None
_jax_err = None

try:
    import jax
    import jax.numpy as jnp

    _IDX = np.arange(K)[:, None] - np.arange(K)[None, :] + K - 1  # [K,K] static

    def _bn(x, g, b, axes):
        m = jnp.mean(x, axes, keepdims=True)
        v = jnp.var(x, axes, keepdims=True)
        sh = (1, -1) + (1,) * (x.ndim - 2)
        return g.reshape(sh) * (x - m) * jax.lax.rsqrt(v + EPS) + b.reshape(sh)

    def _axial(x, qkv_w, bq_g, bq_b, bs_g, bs_b, bo_g, bo_b, rel):
        B = x.shape[0]
        qkv = _bn(jnp.einsum('oc,bcl->bol', qkv_w, x), bq_g, bq_b, (0, 2))
        qkv = qkv.reshape(B, G, 2 * GP, K)
        q, k, v = qkv[:, :, :GP // 2], qkv[:, :, GP // 2:GP], qkv[:, :, GP:]
        emb = rel[:, _IDX]  # [2*GP, K, K]
        q_e, k_e, v_e = emb[:GP // 2], emb[GP // 2:GP], emb[GP:]
        qr = 0.1 * jnp.einsum('bgci,cij->bgij', q, q_e)
        kr = 0.1 * jnp.einsum('bgci,cij->bgji', k, k_e)
        qk = jnp.einsum('bgci,bgcj->bgij', q, k)
        sim = jnp.concatenate([qk, qr, kr], axis=1)          # [B, 3G, K, K]
        sim = _bn(sim, bs_g, bs_b, (0, 2, 3)).reshape(B, 3, G, K, K).sum(1)
        sim = jax.nn.softmax(sim, axis=-1)
        sv = jnp.einsum('bgij,bgcj->bgci', sim, v)
        sve = 0.1 * jnp.einsum('bgij,cij->bgci', sim, v_e)
        out = jnp.concatenate([sv, sve], axis=1).reshape(B, 2 * MID, K)
        out = _bn(out, bo_g, bo_b, (0, 2))
        return out.reshape(B, MID, 2, K).sum(2)              # [B, MID, K]

    def _block(d):
        x = d['x']
        x2 = x.reshape(NB, CIN, K * K)
        x_out = (jnp.einsum('oc,ncl->nol', d['c1_w'], x2)
                 + d['c1_b'][None, :, None]).reshape(NB, COUT, K, K)
        out = (jnp.einsum('oc,ncl->nol', d['cd_w'], x2)
               + d['cd_b'][None, :, None]).reshape(NB, MID, K, K)
        out = jax.nn.relu(_bn(out, d['bn1_g'], d['bn1_b'], (0, 2, 3)))
        # height block: [N, W, C, H] -> [N*W, C, H]
        h_in = out.transpose(0, 3, 1, 2).reshape(NB * K, MID, K)
        h = _axial(h_in, d['h_qkv_w'], d['h_bq_g'], d['h_bq_b'],
                   d['h_bs_g'], d['h_bs_b'], d['h_bo_g'], d['h_bo_b'],
                   d['h_rel'])
        h = h.reshape(NB, K, MID, K).transpose(0, 2, 3, 1)   # [N, C, H, W]
        # width block: [N, H, C, W] -> [N*H, C, W]
        w_in = h.transpose(0, 2, 1, 3).reshape(NB * K, MID, K)
        w = _axial(w_in, d['w_qkv_w'], d['w_bq_g'], d['w_bq_b'],
                   d['w_bs_g'], d['w_bs_b'], d['w_bo_g'], d['w_bo_b'],
                   d['w_rel'])
        w = w.reshape(NB, K, MID, K).transpose(0, 2, 1, 3)   # [N, C, H, W]
        w2 = w.reshape(NB, MID, K * K)
        out = (jnp.einsum('oc,ncl->nol', d['cu_w'], w2)
               + d['cu_b'][None, :, None]).reshape(NB, COUT, K, K)
        out = _bn(out, d['bn2_g'], d['bn2_b'], (0, 2, 3))
        return out + x_out

    def _compile():
        spec = {n: jax.ShapeDtypeStruct(s, jnp.float32) for n, s in _INPUT_SPECS}
        return jax.jit(_block).lower(spec).compile()

    try:
        _compiled = _compile()
    except Exception as e:  # compile lazily on first call instead
        _jax_err = e
except Exception as e:  # jax missing/broken: numpy fallback below
    _jax_err = e


# ---------------------------------------------------------------------------
# numpy fallback (slow, correctness safety net)
# ---------------------------------------------------------------------------

def _np_bn(x, g, b, axes):
    m = x.mean(axes, keepdims=True, dtype=np.float32)
    v = x.var(axes, keepdims=True, dtype=np.float32)
    sh = (1, -1) + (1,) * (x.ndim - 2)
    return (g.reshape(sh) * (x - m) / np.sqrt(v + np.float32(EPS))
            + b.reshape(sh)).astype(np.float32)


def _np_axial(x, qkv_w, bq_g, bq_b, bs_g, bs_b, bo_g, bo_b, rel):
    B = x.shape[0]
    qkv = _np_bn(np.einsum('oc,bcl->bol', qkv_w, x, optimize=True),
                 bq_g, bq_b, (0, 2))
    qkv = qkv.reshape(B, G, 2 * GP, K)
    q, k, v = qkv[:, :, :GP // 2], qkv[:, :, GP // 2:GP], qkv[:, :, GP:]
    idx = np.arange(K)[:, None] - np.arange(K)[None, :] + K - 1
    emb = rel[:, idx]
    q_e, k_e, v_e = emb[:GP // 2], emb[GP // 2:GP], emb[GP:]
    qr = 0.1 * np.einsum('bgci,cij->bgij', q, q_e, optimize=True)
    kr = 0.1 * np.einsum('bgci,cij->bgji', k, k_e, optimize=True)
    qk = np.einsum('bgci,bgcj->bgij', q, k, optimize=True)
    sim = np.concatenate([qk, qr, kr], axis=1).astype(np.float32)
    sim = _np_bn(sim, bs_g, bs_b, (0, 2, 3)).reshape(B, 3, G, K, K).sum(1)
    sim = sim - sim.max(-1, keepdims=True)
    np.exp(sim, out=sim)
    sim /= sim.sum(-1, keepdims=True)
    sv = np.einsum('bgij,bgcj->bgci', sim, v, optimize=True)
    sve = 0.1 * np.einsum('bgij,cij->bgci', sim, v_e, optimize=True)
    out = np.concatenate([sv, sve], axis=1).reshape(B, 2 * MID, K)
    out = _np_bn(out.astype(np.float32), bo_g, bo_b, (0, 2))
    return out.reshape(B, MID, 2, K).sum(2).astype(np.float32)


def _np_block(d):
    x2 = d['x'].reshape(NB, CIN, K * K)
    x_out = (np.einsum('oc,ncl->nol', d['c1_w'], x2, optimize=True)
             + d['c1_b'][None, :, None]).reshape(NB, COUT, K, K)
    out = (np.einsum('oc,ncl->nol', d['cd_w'], x2, optimize=True)
           + d['cd_b'][None, :, None]).reshape(NB, MID, K, K)
    out = np.maximum(_np_bn(out, d['bn1_g'], d['bn1_b'], (0, 2, 3)), 0)
    h_in = out.transpose(0, 3, 1, 2).reshape(NB * K, MID, K)
    h = _np_axial(h_in, d['h_qkv_w'], d['h_bq_g'], d['h_bq_b'],
                  d['h_bs_g'], d['h_bs_b'], d['h_bo_g'], d['h_bo_b'],
                  d['h_rel'])
    h = h.reshape(NB, K, MID, K).transpose(0, 2, 3, 1)
    w_in = np.ascontiguousarray(h.transpose(0, 2, 1, 3)).reshape(NB * K, MID, K)
    w = _np_axial(w_in, d['w_qkv_w'], d['w_bq_g'], d['w_bq_b'],
                  d['w_bs_g'], d['w_bs_b'], d['w_bo_g'], d['w_bo_b'],
                  d['w_rel'])
    w = w.reshape(NB, K, MID, K).transpose(0, 2, 1, 3)
    w2 = np.ascontiguousarray(w).reshape(NB, MID, K * K)
    out = (np.einsum('oc,ncl->nol', d['cu_w'], w2, optimize=True)
           + d['cu_b'][None, :, None]).reshape(NB, COUT, K, K)
    out = _np_bn(out, d['bn2_g'], d['bn2_b'], (0, 2, 3))
    return (out + x_out).astype(np.float32)


def kernel(**inputs):
    global _compiled
    d = {n: np.asarray(inputs[n], dtype=np.float32) for n, _ in _INPUT_SPECS}
    if _compiled is None and _jax_err is not None:
        try:
            _compiled = _compile()
        except Exception:
            _compiled = None
    if _compiled is not None:
        try:
            out = _compiled(d)
            return np.asarray(out, dtype=np.float32)
        except Exception:
            pass
    return _np_block(d)
